# revision 17
# baseline (speedup 1.0000x reference)
"""3-layer GAT (graph attention) on 8 trn2 NeuronCores.

Strategy: shard destination nodes across cores (6250/core, padded to
6272 rows). Per layer: each core computes its shard of an augmented
table  [h | es | ed | 1 | pad]  (h = y@W, es/ed = per-node attention
terms) in fp16 via TensorE, AllGathers the 50176x132 fp16 table, then
processes its edges (sorted by dst, per-tile subtile schedule shared
across cores):

- per subtile, ONE [128,1]-offset indirect DMA gathers the 128 src
  rows (264B each); es[src] rides along in column 128.
- ed[dst] is NEVER gathered: an ed row (per-node dst attention term) is
  computed directly as (W a_d)^T @ ytc via 1-column matmuls, broadcast
  to all partitions with a ones-matmul, and expanded per-edge with the
  same one-hot(dst) mask used by the scatter (mask*row, reduce-X).
- a one-hot(dst)*exp(logit) fp16 matrix is built per tile in two
  broadcast VectorE ops, and fp16 TensorE matmuls accumulate weighted
  feature sums + softmax denominator per 128-dst tile in f32 PSUM.
- pad slots use dloc=200: the one-hot row is all-zero, so they
  contribute nothing regardless of what the pad gather returns.
"""
import sys
sys.path.insert(0, "/opt/trn_rl_repo")
import numpy as np
import concourse.bass as bass
import concourse.mybir as mybir
import concourse.tile as tile
from concourse.bass_utils import run_bass_kernel_spmd

N = 50000
E = 600000
F_IN, HID, F_OUT = 64, 128, 64
NC_ = 8
SH = 6250          # real nodes per shard
SHP = 6272         # padded shard rows (49 * 128)
NT = NC_ * SHP     # table rows
NTILE = SHP // 128 # 49 dst tiles per core
TW = 132           # table row width: h(128), es, ed, one, pad
ECH = 512          # ed-row chunk (PSUM bank width in f32)
NECH = SHP // ECH  # 12.25 -> handled with a remainder chunk
NEG_ATT = 0.2
NEG_ACT = 0.01
DUMMY = 0          # table row gathered by pad slots (any valid row)
PADLOC = 200.0     # dloc for pad slots: no one-hot match -> zero contribution

f32 = mybir.dt.float32
f16 = mybir.dt.float16
i32 = mybir.dt.int32


def _hoist_waits(nc):
    """walrus on this toolchain allows only ONE sync-wait slot per
    instruction; move extras onto preceding same-engine NoOps."""
    for fn in nc.m.functions:
        for blk in fn.blocks:
            new_insts = []
            for inst in blk.instructions:
                si = getattr(inst, "sync_info", None)
                waits = list(si.on_wait) if si is not None and si.on_wait else []
                if len(waits) > 1:
                    keep, extra = waits[:1], waits[1:]
                    while extra:
                        chunk, extra = extra[:1], extra[1:]
                        nop = mybir.InstNoOp(
                            name=nc.get_next_instruction_name(),
                            engine=inst.engine, bass_nofuse=True)
                        nop.sync_info = mybir.SyncInfo(on_wait=chunk, on_update=[])
                        new_insts.append(nop)
                    inst.sync_info = mybir.SyncInfo(
                        on_wait=keep,
                        on_update=list(si.on_update) if si.on_update else [])
                new_insts.append(inst)
            blk.instructions = new_insts


def _build(cd):
    cd = list(cd)
    kmax = max(cd)
    nsub = sum(cd)
    starts = np.zeros(NTILE, np.int64)
    starts[1:] = np.cumsum(cd)[:-1]
    # ed-row chunks over the SHP node axis
    chunks = []
    off = 0
    while off < SHP:
        w = min(ECH, SHP - off)
        chunks.append((off, w))
        off += w

    nc = bass.Bass()
    xT = nc.declare_dram_parameter("xT", [F_IN, SHP], f32, isOutput=False)
    osrc = nc.declare_dram_parameter("osrc", [128, nsub], i32, isOutput=False)
    dloc = nc.declare_dram_parameter("dloc", [128, nsub], f16, isOutput=False)
    waug = [nc.declare_dram_parameter(f"waug{l}", [128, 130], f32, isOutput=False)
            for l in range(3)]
    wout = nc.declare_dram_parameter("wout", [128, F_OUT], f32, isOutput=False)
    bb = [nc.declare_dram_parameter(f"bb{l}", [128, 128], f32, isOutput=False)
          for l in range(3)]
    bbo = nc.declare_dram_parameter("bbo", [128, F_OUT], f32, isOutput=False)
    iota = nc.declare_dram_parameter("iota", [128, 128], f16, isOutput=False)
    ident = nc.declare_dram_parameter("ident", [128, 128], f32, isOutput=False)
    out_e = nc.declare_dram_parameter("out", [SHP, F_OUT], f32, isOutput=True)

    tbl_in = [nc.dram_tensor(f"tbl_in{l}", [SHP, TW], f16) for l in range(3)]
    tblJ = [nc.dram_tensor(f"tbl{l}", [NT, TW], f16, addr_space="Shared")
            for l in range(3)]
    SEG_T = [0, 7, 13, 20, 26, 33, 39, 44, 49]  # tile bounds of the AG segments
    RB = [t * 128 for t in SEG_T]          # row bounds within a shard

    with tile.TileContext(nc) as tc:
        with (
            tc.tile_pool(name="const", bufs=1) as cpool,
            tc.tile_pool(name="stage", bufs=6) as spool,
            tc.tile_pool(name="gat", bufs=2 * kmax + 4) as gpool,
            tc.tile_pool(name="edf", bufs=3) as efpool,
            tc.tile_pool(name="lgt", bufs=6) as lpool,
            tc.tile_pool(name="bx", bufs=4) as bxpool,
            tc.tile_pool(name="edm", bufs=3) as empool,
            tc.tile_pool(name="epi", bufs=6) as tpool,
            tc.tile_pool(name="ps_h", bufs=2, space="PSUM") as ps_h,
            tc.tile_pool(name="ps_nm", bufs=2, space="PSUM") as ps_nm,
            tc.tile_pool(name="ps_t", bufs=1, space="PSUM") as ps_t,
            tc.tile_pool(name="ps_e", bufs=1, space="PSUM") as ps_e,
            tc.tile_pool(name="ps_r", bufs=1, space="PSUM") as ps_r,
        ):
            # ---- resident constants / inputs
            osrc_sb = cpool.tile([128, nsub], i32)
            dloc_sb = cpool.tile([128, nsub], f16)
            nc.sync.dma_start(out=osrc_sb[:], in_=osrc[:])
            nc.sync.dma_start(out=dloc_sb[:], in_=dloc[:])
            waug_sb = [cpool.tile([128, 130], f32, name=f"waug_sb{l}") for l in range(3)]
            for l in range(3):
                nc.sync.dma_start(out=waug_sb[l][:], in_=waug[l][:])
            wout_sb = cpool.tile([128, F_OUT], f32)
            nc.sync.dma_start(out=wout_sb[:], in_=wout[:])
            bb_sb = [cpool.tile([128, 128], f32, name=f"bb_sb{l}") for l in range(3)]
            for l in range(3):
                nc.sync.dma_start(out=bb_sb[l][:], in_=bb[l][:])
            bbo_sb = cpool.tile([128, F_OUT], f32)
            nc.sync.dma_start(out=bbo_sb[:], in_=bbo[:])
            iota_sb = cpool.tile([128, 128], f16)
            nc.sync.dma_start(out=iota_sb[:], in_=iota[:])
            id_sb = cpool.tile([128, 128], f32)
            nc.sync.dma_start(out=id_sb[:], in_=ident[:])
            ones1_sb = cpool.tile([1, 128], f16)
            nc.vector.memset(ones1_sb[:], 1.0)
            edrep_sb = [cpool.tile([128, SHP], f16, name=f"edrep{i}")
                        for i in range(2)]   # ed per node, all partitions

            # YT double buffer: layer input, feat x nodes (feat on partitions)
            yt_sb = [cpool.tile([128, SHP], f32, name=f"yt{i}") for i in range(2)]
            nc.sync.dma_start(out=yt_sb[0][:F_IN, :], in_=xT[:])

            def phase1_tile(l, t):
                fin = F_IN if l == 0 else HID
                ytc = yt_sb[l % 2]
                hps = ps_h.tile([128, 130], f32, tag="hps", name="hps")
                nc.tensor.matmul(hps[:], lhsT=ytc[:fin, t*128:(t+1)*128],
                                 rhs=waug_sb[l][:fin, :], start=True, stop=True)
                stg = spool.tile([128, TW], f16, tag="stg", name="stg")
                nc.scalar.activation(stg[:, 0:130], hps[:],
                                     mybir.ActivationFunctionType.Copy)
                nc.vector.memset(stg[:, 130:131], 1.0)
                nc.vector.memset(stg[:, 131:132], 0.0)
                nc.sync.dma_start(out=tbl_in[l][t*128:(t+1)*128, :], in_=stg[:])

            def edrep_chunk(l, coff, cw):
                fin = F_IN if l == 0 else HID
                ytc = yt_sb[l % 2]
                efp = ps_e.tile([1, ECH], f32, tag="efp", name="efp")
                nc.tensor.matmul(efp[:, :cw],
                                 lhsT=waug_sb[l][:fin, 129:130],
                                 rhs=ytc[:fin, coff:coff+cw],
                                 start=True, stop=True)
                efs = efpool.tile([1, ECH], f16, tag="efs", name="efs")
                nc.vector.tensor_copy(out=efs[:, :cw], in_=efp[:, :cw])
                erp = ps_r.tile([128, ECH], f32, tag="erp", name="erp")
                nc.tensor.matmul(erp[:, :cw], lhsT=ones1_sb[:],
                                 rhs=efs[:, :cw], start=True, stop=True)
                nc.vector.tensor_copy(out=edrep_sb[l % 2][:, coff:coff+cw],
                                      in_=erp[:, :cw])

            SEG_LAST = [SEG_T[i+1] - 1 for i in range(len(SEG_T)-1)]
            def allgather_seg(l, i):
                nc.gpsimd.collective_compute(
                    "AllGather", mybir.AluOpType.bypass,
                    replica_groups=[list(range(NC_))],
                    ins=[tbl_in[l][RB[i]:RB[i+1], :]],
                    outs=[tblJ[l][NC_*RB[i]:NC_*RB[i+1], :]])

            # layer 0 front matter
            for t in range(NTILE):
                phase1_tile(0, t)
                if t in SEG_LAST:
                    allgather_seg(0, SEG_LAST.index(t))
            for (coff, cw) in chunks:
                edrep_chunk(0, coff, cw)

            for l in range(3):
                ytn = yt_sb[(l + 1) % 2]
                edrep_cur = edrep_sb[l % 2]

                # edrep chunks of layer l+1 emitted once their ytn tiles land
                pending = {(c if cw == ECH else len(chunks) - 1):
                           (coff, cw) for c, (coff, cw) in enumerate(chunks)}
                def maybe_next_layer(d):
                    if l >= 2:
                        return
                    phase1_tile(l + 1, d)
                    for c, (coff, cw) in enumerate(chunks):
                        if (coff, cw) in pending.values() and \
                           (coff + cw + 127) // 128 - 1 == d:
                            edrep_chunk(l + 1, coff, cw)
                            pending.pop(c, None)
                    if d in SEG_LAST:
                        allgather_seg(l + 1, SEG_LAST.index(d))

                # ---- phase 3: edges, per dst tile
                for d in range(NTILE):
                    k = cd[d]
                    j0 = int(starts[d])
                    gts = []
                    for s in range(k):
                        g = gpool.tile([128, TW], f16, tag="G", name="g")
                        if s == 0:
                            # self-loop subtile: the tile's own local rows
                            nc.sync.dma_start(
                                out=g[:], in_=tbl_in[l][d*128:(d+1)*128, :])
                        else:
                            srow = osrc_sb[:, j0+s:j0+s+1]
                            nc.gpsimd.indirect_dma_start(
                                out=g[:], out_offset=None, in_=tblJ[l][:],
                                in_offset=bass.IndirectOffsetOnAxis(
                                    ap=srow, axis=0))
                        gts.append(g)
                    # one-hot(dst) for all k subtiles in one broadcast op
                    bxt = bxpool.tile([128, kmax * 128], f16, tag="bxt", name="bxt")
                    b3 = bxt[:, 0:k*128].rearrange("p (k c) -> p k c", k=k)
                    nc.vector.tensor_tensor(
                        out=b3,
                        in0=iota_sb[:].unsqueeze(1).broadcast_to((128, k, 128)),
                        in1=dloc_sb[:, j0:j0+k].unsqueeze(2).broadcast_to(
                            (128, k, 128)),
                        op=mybir.AluOpType.is_equal)
                    # ed per edge = reduce(one-hot * ed_row_tile)
                    edm = empool.tile([128, kmax * 128], f16, tag="edm", name="edm")
                    e3 = edm[:, 0:k*128].rearrange("p (k c) -> p k c", k=k)
                    nc.vector.tensor_tensor(
                        out=e3, in0=b3,
                        in1=edrep_cur[:, d*128:(d+1)*128].unsqueeze(1)
                            .broadcast_to((128, k, 128)),
                        op=mybir.AluOpType.mult)
                    ede = lpool.tile([128, kmax], f32, tag="ede", name="ede")
                    nc.vector.tensor_reduce(out=ede[:, :k], in_=e3,
                                            axis=mybir.AxisListType.X,
                                            op=mybir.AluOpType.add)
                    # logits -> exp  (es gathered in column 128 of each row)
                    esb = lpool.tile([128, kmax], f32, tag="esb", name="esb")
                    etm = lpool.tile([128, kmax], f32, tag="etm", name="etm")
                    exb = lpool.tile([128, kmax], f16, tag="exb", name="exb")
                    for s in range(k):
                        nc.vector.tensor_tensor(
                            out=esb[:, s:s+1], in0=gts[s][:, 128:129],
                            in1=ede[:, s:s+1], op=mybir.AluOpType.add)
                    nc.vector.tensor_scalar_mul(etm[:, :k], esb[:, :k], NEG_ATT)
                    nc.vector.tensor_tensor(out=esb[:, :k], in0=esb[:, :k],
                                            in1=etm[:, :k],
                                            op=mybir.AluOpType.max)
                    nc.scalar.activation(exb[:, :k], esb[:, :k],
                                         mybir.ActivationFunctionType.Exp)
                    # bx = one-hot * exp(e), all k subtiles in one op
                    nc.vector.tensor_tensor(
                        out=b3, in0=b3,
                        in1=exb[:, 0:k].unsqueeze(2).broadcast_to((128, k, 128)),
                        op=mybir.AluOpType.mult)
                    nmps = ps_nm.tile([128, 131], f32, tag="nm", name="nmps")
                    for s in range(k):
                        nc.tensor.matmul(nmps[:], lhsT=bxt[:, s*128:(s+1)*128],
                                         rhs=gts[s][:, 0:131],
                                         start=(s == 0), stop=(s == k - 1))
                    # ---- epilogue: y = numer/denom + b, activation, transpose
                    dn = tpool.tile([128, 1], f32, tag="dn", name="dn")
                    nc.vector.tensor_scalar_add(dn[:], nmps[:, 130:131], 1e-16)
                    rec = tpool.tile([128, 1], f32, tag="rec", name="rec")
                    nc.vector.reciprocal(rec[:], dn[:])
                    y = tpool.tile([128, 128], f32, tag="y", name="y")
                    nc.vector.tensor_scalar(
                        out=y[:], in0=nmps[:, 0:128], scalar1=rec[:],
                        scalar2=None, op0=mybir.AluOpType.mult)
                    nc.vector.tensor_tensor(out=y[:], in0=y[:], in1=bb_sb[l][:],
                                            op=mybir.AluOpType.add)
                    y2 = tpool.tile([128, 128], f32, tag="y2", name="y2")
                    nc.vector.tensor_scalar_mul(y2[:], y[:], NEG_ACT)
                    nc.vector.tensor_tensor(out=y[:], in0=y[:], in1=y2[:],
                                            op=mybir.AluOpType.max)
                    tps = ps_t.tile([128, 128], f32, tag="tps", name="tps")
                    nc.tensor.transpose(tps[:], y[:], id_sb[:])
                    if l < 2:
                        nc.vector.tensor_copy(out=ytn[:, d*128:(d+1)*128], in_=tps[:])
                    else:
                        y3t = tpool.tile([128, 128], f32, tag="y3t", name="y3t")
                        nc.vector.tensor_copy(out=y3t[:], in_=tps[:])
                        ops = ps_t.tile([128, F_OUT], f32, tag="ops", name="ops")
                        nc.tensor.matmul(ops[:], lhsT=y3t[:], rhs=wout_sb[:],
                                         start=True, stop=True)
                        ot = tpool.tile([128, F_OUT], f32, tag="ot", name="ot")
                        nc.vector.tensor_tensor(out=ot[:], in0=ops[:], in1=bbo_sb[:],
                                                op=mybir.AluOpType.add)
                        nc.sync.dma_start(out=out_e[d*128:(d+1)*128, :], in_=ot[:])
                    maybe_next_layer(d)
    _hoist_waits(nc)
    return nc


_CACHE = {}
LAST = None  # last BassKernelResults (for test harness introspection)


def _prep(x, edge_index, W0, as0, ad0, b0, W1, as1, ad1, b1,
          W2, as2, ad2, b2, Wout, bout):
    x = np.asarray(x, np.float32)
    ei = np.asarray(edge_index)
    src = np.concatenate([ei[0], np.arange(N, dtype=np.int64)]).astype(np.int64)
    dst = np.concatenate([ei[1], np.arange(N, dtype=np.int64)]).astype(np.int64)

    core = dst // SH
    # table layout after four rank-concatenated AllGather segments with
    # shard-row bounds RB: segment i holds [c0 rows RB[i]..RB[i+1]), c1 .., c7]
    RB = np.array([0, 7*128, 13*128, 20*128, 26*128, 33*128,
                   39*128, 44*128, 49*128], np.int64)
    sc = src // SH
    sr = src % SH
    seg = np.searchsorted(RB, sr, side="right") - 1
    segw = RB[seg+1] - RB[seg]
    row_of_src = NC_ * RB[seg] + sc * segw + (sr - RB[seg])
    # appended self-loops (indices >= E+... actually last N entries) get a
    # dedicated, direct-loaded subtile 0 per tile; only graph edges are slotted
    is_graph = np.zeros(len(src), bool)
    is_graph[:E] = True

    per_core = []
    cnts = np.zeros((NC_, NTILE), np.int64)
    for r in range(NC_):
        m = (core == r) & is_graph
        s_r = row_of_src[m]
        d_r = dst[m] - r * SH           # local dst 0..6249
        o = np.argsort(d_r, kind="stable")
        s_r, d_r = s_r[o], d_r[o]
        t_r = d_r // 128                # dst tile
        cnt = np.bincount(t_r, minlength=NTILE)
        cnts[r] = cnt
        per_core.append((s_r, d_r, t_r, cnt))

    # per-tile subtile counts (1 self subtile + graph subtiles), shared
    # across cores (SPMD program uniformity)
    cg = np.maximum(1, np.ceil(cnts.max(axis=0) / 128.0).astype(np.int64))
    cd = cg + 1
    starts = np.zeros(NTILE, np.int64)
    starts[1:] = np.cumsum(cd)[:-1]
    nsub = int(cd.sum())

    in_maps = []
    for r in range(NC_):
        s_r, d_r, t_r, cnt = per_core[r]
        osrc = np.full((128, nsub), DUMMY, np.int32)
        dloc = np.full((128, nsub), PADLOC, np.float16)
        # self subtile: lane == local dst; pad lanes of the last tile stay PADLOC
        for t in range(NTILE):
            nreal = min(128, SH - t * 128)
            dloc[:nreal, starts[t]] = np.arange(nreal, dtype=np.float16)
        tstart = np.zeros(NTILE, np.int64)
        tstart[1:] = np.cumsum(cnt)[:-1]
        kk = np.arange(len(d_r)) - tstart[t_r]     # rank within dst tile
        sub = starts[t_r] + 1 + (kk // 128)        # subtile slot (0 = self)
        lane = kk % 128
        osrc[lane, sub] = s_r
        dloc[lane, sub] = (d_r % 128).astype(np.float16)

        xT = np.zeros((F_IN, SHP), np.float32)
        xT[:, :SH] = x[r*SH:(r+1)*SH].T
        in_maps.append({"xT": xT, "osrc": osrc, "dloc": dloc})

    def aug(W, a_s, a_d):
        W = np.asarray(W, np.float32)
        out = np.zeros((128, 130), np.float32)
        out[:W.shape[0], :128] = W
        out[:W.shape[0], 128] = W @ np.asarray(a_s, np.float32)
        out[:W.shape[0], 129] = W @ np.asarray(a_d, np.float32)
        return out

    shared = {
        "waug0": aug(W0, as0, ad0), "waug1": aug(W1, as1, ad1),
        "waug2": aug(W2, as2, ad2),
        "wout": np.asarray(Wout, np.float32),
        "bb0": np.tile(np.asarray(b0, np.float32), (128, 1)),
        "bb1": np.tile(np.asarray(b1, np.float32), (128, 1)),
        "bb2": np.tile(np.asarray(b2, np.float32), (128, 1)),
        "bbo": np.tile(np.asarray(bout, np.float32), (128, 1)),
        "iota": np.tile(np.arange(128, dtype=np.float16), (128, 1)),
        "ident": np.eye(128, dtype=np.float32),
    }
    for m in in_maps:
        m.update(shared)
    return tuple(cd.tolist()), in_maps


def kernel(**inputs):
    global LAST
    cd, in_maps = _prep(**inputs)
    if cd not in _CACHE:
        _CACHE[cd] = _build(cd)
    nc = _CACHE[cd]
    res = run_bass_kernel_spmd(nc, in_maps, list(range(NC_)))
    LAST = res
    return np.concatenate([res.results[r]["out"][:SH] for r in range(NC_)], axis=0)
